# revision 8
# baseline (speedup 1.0000x reference)
"""Trainium2 kernel for nn_CrossAttention_74972949119465.

Math note: the reference tiles x_img [b, 1, 512] across the full sequence
before projecting K and V, so V is identical for every key position.  Since
softmax weights sum to 1, the attention output for every query is exactly
v_row = tile(x_img[b,0],8) @ wv, independent of x/wq/wk/RoPE and any finite
mask.  The module output is therefore

    out[b, s, :] = (tile(x_img[b, 0, :], 8) @ wv) @ wo        for all s.

The device kernel computes exactly that, tensor-parallel over 8 cores:
core c holds the column slice wv[:, 512c:512(c+1)] and the matching row
slice wo[512c:512(c+1), :].  Both GEMMs are laid out with the big weight
matrix as the stationary (LDWEIGHTS) operand and the tiny activation
vector as the moving operand, which keeps full fp32 precision, avoids the
4-cycles-per-row fp32 moving-operand penalty, and produces v_slice already
transposed for the second GEMM.  The host sums the eight [2, 4096]
partials and broadcasts over the sequence dimension.
"""

import numpy as np

BSZ, SEQ, DIM, IMG = 2, 1024, 4096, 512
NCORES = 8
CSLICE = DIM // NCORES  # 512 columns of wv / rows of wo per core
P = 128                 # partitions
KT = DIM // P           # 32 contraction tiles for vin @ wv_c
KT2 = CSLICE // P       # 4 contraction tiles for v_slice @ wo_c
MT = CSLICE // P        # 4 output blocks of v_slice
MT2 = DIM // P          # 32 output blocks of the partial output

_cache = {}


def _build_nc():
    import concourse.bass as bass
    import concourse.mybir as mybir
    import concourse.tile as tile
    from concourse import bacc

    fp32 = mybir.dt.float32
    nc = bacc.Bacc(None, target_bir_lowering=False)

    # vin pre-laid-out on host: vin_d[p, kt*BSZ + m] = vin[m, kt*P + p]
    vin_d = nc.dram_tensor("vin", [P, KT * BSZ], fp32, kind="ExternalInput")
    wv_d = nc.dram_tensor("wv_c", [DIM, CSLICE], fp32, kind="ExternalInput")
    wo_d = nc.dram_tensor("wo_c", [CSLICE, DIM], fp32, kind="ExternalInput")
    # transposed partial: part_t[p, m2*BSZ + m] = part[m, m2*P + p]
    out_d = nc.dram_tensor("part_t", [P, MT2 * BSZ], fp32, kind="ExternalOutput")

    with tile.TileContext(nc) as tc:
        with (
            tc.tile_pool(name="weights", bufs=1) as wpool,
            tc.tile_pool(name="small", bufs=1) as spool,
            tc.tile_pool(name="vps", bufs=1, space=bass.MemorySpace.PSUM) as vpool,
            tc.tile_pool(name="ops", bufs=4, space=bass.MemorySpace.PSUM) as opool,
        ):
            # wv_c: 8 DMAs of 1 MB (4 contiguous [128, 512] k-tiles each);
            # first wv chunk issued before anything else so the weight
            # stream starts as early as possible
            wv_sb = wpool.tile([P, KT, CSLICE], fp32)
            wv_r4 = wv_d[:].rearrange("(t kt p) n -> t p kt n", p=P, kt=4)
            nc.sync.dma_start(wv_sb[:, 0:4, :], wv_r4[0])

            vin_sb = spool.tile([P, KT, BSZ], fp32)
            nc.sync.dma_start(
                vin_sb[:], vin_d[:].rearrange("p (kt m) -> p kt m", m=BSZ)
            )
            for t in range(1, KT // 4):
                nc.sync.dma_start(wv_sb[:, 4 * t:4 * t + 4, :], wv_r4[t])

            # wo_c: column-chunk-outer so stage B's first output group is
            # ready after the first chunk lands; the final chunk is a small
            # 128-column slice so almost no work gates on the last byte
            wo_sb = wpool.tile([P, KT2, DIM], fp32)
            wo_r = wo_d[:].rearrange("(kt p) n -> p kt n", p=P)
            wo_chunks = [(0, 1024), (1024, 1024), (2048, 1024),
                         (3072, 896), (3968, 128)]
            for c0, cw in wo_chunks:
                nc.sync.dma_start(
                    wo_sb[:, :, c0:c0 + cw],
                    wo_r[:, :, c0:c0 + cw],
                )

            # Stage A: vT[p_of_mblock, j, m] = sum_k wv_c[k, j*P+p] * vin[m, k]
            # weights stationary, vin moving -> v_slice lands pre-transposed.
            vT_ps = vpool.tile([P, MT, BSZ], fp32)
            for j in range(MT):
                for kt in range(KT):
                    nc.tensor.matmul(
                        vT_ps[:, j, :],
                        wv_sb[:, kt, j * P:(j + 1) * P],
                        vin_sb[:, kt, :],
                        start=(kt == 0),
                        stop=(kt == KT - 1),
                    )
            vT_sb = spool.tile([P, MT, BSZ], fp32)
            nc.any.tensor_copy(vT_sb[:], vT_ps[:])

            # Stage B: partT[p, m2, m] = sum_k wo_c[k, m2*P+p] * v_slice[m, k]
            # grouped to match the wo DMA chunks; each group copies its PSUM
            # blocks to SBUF and DMAs them out immediately, so only the tiny
            # last group (one block) chains after the final wo byte arrives.
            oT_sb = spool.tile([P, MT2, BSZ], fp32)
            out_r = out_d[:].rearrange("p (m2 m) -> p m2 m", m=BSZ)
            groups = [(0, 8), (8, 8), (16, 8), (24, 8)]
            for g0, gn in groups:
                oT_ps = opool.tile([P, 8, BSZ], fp32)
                for mi in range(gn):
                    m2 = g0 + mi
                    for kt in range(KT2):
                        nc.tensor.matmul(
                            oT_ps[:, mi, :],
                            wo_sb[:, kt, m2 * P:(m2 + 1) * P],
                            vT_sb[:, kt, :],
                            start=(kt == 0),
                            stop=(kt == KT2 - 1),
                        )
                nc.any.tensor_copy(
                    oT_sb[:, g0:g0 + gn, :], oT_ps[:, :gn, :]
                )
                nc.sync.dma_start(
                    out_r[:, g0:g0 + gn, :], oT_sb[:, g0:g0 + gn, :]
                )

    nc.compile()
    return nc


def _make_in_maps(inputs):
    x_img = np.asarray(inputs["x_img"], dtype=np.float32)
    wv = np.asarray(inputs["wv"], dtype=np.float32)
    wo = np.asarray(inputs["wo"], dtype=np.float32)

    vin = np.tile(x_img[:, 0, :], (1, DIM // IMG))  # [2, 4096]
    vin_dev = np.ascontiguousarray(
        vin.T.reshape(KT, P, BSZ).transpose(1, 0, 2).reshape(P, KT * BSZ)
    )
    in_maps = []
    for c in range(NCORES):
        in_maps.append({
            "vin": vin_dev,
            "wv_c": np.ascontiguousarray(wv[:, c * CSLICE:(c + 1) * CSLICE]),
            "wo_c": np.ascontiguousarray(wo[c * CSLICE:(c + 1) * CSLICE, :]),
        })
    return in_maps


def _run(inputs, trace=False, trace_cores=None):
    from concourse.bass_utils import run_bass_kernel_spmd

    if "nc" not in _cache:
        _cache["nc"] = _build_nc()
    nc = _cache["nc"]

    in_maps = _make_in_maps(inputs)
    core_ids = list(range(NCORES))
    try:
        res = run_bass_kernel_spmd(
            nc, in_maps, core_ids=core_ids, trace=trace, trace_cores=trace_cores
        )
    except ModuleNotFoundError:
        # BASS_TRACE=1 without the axon NTFF hook module raises before
        # execution; retry untraced rather than failing the run.
        import os

        os.environ["BASS_NEVER_TRACE"] = "1"
        res = run_bass_kernel_spmd(nc, in_maps, core_ids=core_ids)
    o = np.zeros((BSZ, DIM), np.float32)
    for r in res.results:
        part_t = r["part_t"].reshape(P, MT2, BSZ)
        o += part_t.transpose(2, 1, 0).reshape(BSZ, DIM)
    out = np.ascontiguousarray(
        np.broadcast_to(o[:, None, :], (BSZ, SEQ, DIM))
    ).astype(np.float32, copy=False)
    return out, res


def kernel(**inputs):
    out, _ = _run(inputs)
    return out


# revision 9
# speedup vs baseline: 1.0022x; 1.0022x over previous
"""Trainium2 kernel for nn_CrossAttention_74972949119465.

Math note: the reference tiles x_img [b, 1, 512] across the full sequence
before projecting K and V, so V is identical for every key position.  Since
softmax weights sum to 1, the attention output for every query is exactly
v_row = tile(x_img[b,0],8) @ wv, independent of x/wq/wk/RoPE and any finite
mask.  The module output is therefore

    out[b, s, :] = (tile(x_img[b, 0, :], 8) @ wv) @ wo        for all s.

The device kernel computes exactly that, tensor-parallel over 8 cores:
core c holds the column slice wv[:, 512c:512(c+1)] and the matching row
slice wo[512c:512(c+1), :].  Both GEMMs are laid out with the big weight
matrix as the stationary (LDWEIGHTS) operand and the tiny activation
vector as the moving operand, which keeps full fp32 precision, avoids the
4-cycles-per-row fp32 moving-operand penalty, and produces v_slice already
transposed for the second GEMM.  The host sums the eight [2, 4096]
partials and broadcasts over the sequence dimension.
"""

import numpy as np

BSZ, SEQ, DIM, IMG = 2, 1024, 4096, 512
NCORES = 8
CSLICE = DIM // NCORES  # 512 columns of wv / rows of wo per core
P = 128                 # partitions
KT = DIM // P           # 32 contraction tiles for vin @ wv_c
KT2 = CSLICE // P       # 4 contraction tiles for v_slice @ wo_c
MT = CSLICE // P        # 4 output blocks of v_slice
MT2 = DIM // P          # 32 output blocks of the partial output

_cache = {}


def _build_nc():
    import concourse.bass as bass
    import concourse.mybir as mybir
    import concourse.tile as tile
    from concourse import bacc

    fp32 = mybir.dt.float32
    nc = bacc.Bacc(None, target_bir_lowering=False)

    # vin pre-laid-out on host: vin_d[p, kt*BSZ + m] = vin[m, kt*P + p]
    vin_d = nc.dram_tensor("vin", [P, KT * BSZ], fp32, kind="ExternalInput")
    wv_d = nc.dram_tensor("wv_c", [DIM, CSLICE], fp32, kind="ExternalInput")
    wo_d = nc.dram_tensor("wo_c", [CSLICE, DIM], fp32, kind="ExternalInput")
    # transposed partial: part_t[p, m2*BSZ + m] = part[m, m2*P + p]
    out_d = nc.dram_tensor("part_t", [P, MT2 * BSZ], fp32, kind="ExternalOutput")

    with tile.TileContext(nc) as tc:
        with (
            tc.tile_pool(name="weights", bufs=1) as wpool,
            tc.tile_pool(name="small", bufs=1) as spool,
            tc.tile_pool(name="vps", bufs=1, space=bass.MemorySpace.PSUM) as vpool,
            tc.tile_pool(name="ops", bufs=4, space=bass.MemorySpace.PSUM) as opool,
        ):
            # wv_c: 8 DMAs of 1 MB (4 contiguous [128, 512] k-tiles each);
            # first wv chunk issued before anything else so the weight
            # stream starts as early as possible
            wv_sb = wpool.tile([P, KT, CSLICE], fp32)
            wv_r4 = wv_d[:].rearrange("(t kt p) n -> t p kt n", p=P, kt=4)
            nc.sync.dma_start(wv_sb[:, 0:4, :], wv_r4[0])

            vin_sb = spool.tile([P, KT, BSZ], fp32)
            nc.sync.dma_start(
                vin_sb[:], vin_d[:].rearrange("p (kt m) -> p kt m", m=BSZ)
            )
            for t in range(1, KT // 4):
                nc.sync.dma_start(wv_sb[:, 4 * t:4 * t + 4, :], wv_r4[t])

            # wo_c: column-chunk-outer so stage B's first output group is
            # ready after the first chunk lands; the final chunk is a small
            # 128-column slice so almost no work gates on the last byte
            wo_sb = wpool.tile([P, KT2, DIM], fp32)
            wo_r = wo_d[:].rearrange("(kt p) n -> p kt n", p=P)
            wo_chunks = [(0, 1024), (1024, 1024), (2048, 1024),
                         (3072, 896), (3968, 128)]
            for c0, cw in wo_chunks:
                nc.sync.dma_start(
                    wo_sb[:, :, c0:c0 + cw],
                    wo_r[:, :, c0:c0 + cw],
                )

            # Stage A: vT[p_of_mblock, j, m] = sum_k wv_c[k, j*P+p] * vin[m, k]
            # weights stationary, vin moving -> v_slice lands pre-transposed.
            vT_ps = vpool.tile([P, MT, BSZ], fp32)
            for j in range(MT):
                for kt in range(KT):
                    nc.tensor.matmul(
                        vT_ps[:, j, :],
                        wv_sb[:, kt, j * P:(j + 1) * P],
                        vin_sb[:, kt, :],
                        start=(kt == 0),
                        stop=(kt == KT - 1),
                    )
            vT_sb = spool.tile([P, MT, BSZ], fp32)
            nc.vector.tensor_copy(vT_sb[:], vT_ps[:])

            # Stage B: partT[p, m2, m] = sum_k wo_c[k, m2*P+p] * v_slice[m, k]
            # grouped to match the wo DMA chunks; each group copies its PSUM
            # blocks to SBUF and DMAs them out immediately, so only the tiny
            # last group (one block) chains after the final wo byte arrives.
            oT_sb = spool.tile([P, MT2, BSZ], fp32)
            out_r = out_d[:].rearrange("p (m2 m) -> p m2 m", m=BSZ)
            groups = [(0, 8), (8, 8), (16, 8), (24, 8)]
            for g0, gn in groups:
                oT_ps = opool.tile([P, 8, BSZ], fp32)
                for mi in range(gn):
                    m2 = g0 + mi
                    for kt in range(KT2):
                        nc.tensor.matmul(
                            oT_ps[:, mi, :],
                            wo_sb[:, kt, m2 * P:(m2 + 1) * P],
                            vT_sb[:, kt, :],
                            start=(kt == 0),
                            stop=(kt == KT2 - 1),
                        )
                nc.vector.tensor_copy(
                    oT_sb[:, g0:g0 + gn, :], oT_ps[:, :gn, :]
                )
                nc.sync.dma_start(
                    out_r[:, g0:g0 + gn, :], oT_sb[:, g0:g0 + gn, :]
                )

    nc.compile()
    return nc


def _make_in_maps(inputs):
    x_img = np.asarray(inputs["x_img"], dtype=np.float32)
    wv = np.asarray(inputs["wv"], dtype=np.float32)
    wo = np.asarray(inputs["wo"], dtype=np.float32)

    vin = np.tile(x_img[:, 0, :], (1, DIM // IMG))  # [2, 4096]
    vin_dev = np.ascontiguousarray(
        vin.T.reshape(KT, P, BSZ).transpose(1, 0, 2).reshape(P, KT * BSZ)
    )
    in_maps = []
    for c in range(NCORES):
        in_maps.append({
            "vin": vin_dev,
            "wv_c": np.ascontiguousarray(wv[:, c * CSLICE:(c + 1) * CSLICE]),
            "wo_c": np.ascontiguousarray(wo[c * CSLICE:(c + 1) * CSLICE, :]),
        })
    return in_maps


def _run(inputs, trace=False, trace_cores=None):
    from concourse.bass_utils import run_bass_kernel_spmd

    if "nc" not in _cache:
        _cache["nc"] = _build_nc()
    nc = _cache["nc"]

    in_maps = _make_in_maps(inputs)
    core_ids = list(range(NCORES))
    try:
        res = run_bass_kernel_spmd(
            nc, in_maps, core_ids=core_ids, trace=trace, trace_cores=trace_cores
        )
    except ModuleNotFoundError:
        # BASS_TRACE=1 without the axon NTFF hook module raises before
        # execution; retry untraced rather than failing the run.
        import os

        os.environ["BASS_NEVER_TRACE"] = "1"
        res = run_bass_kernel_spmd(nc, in_maps, core_ids=core_ids)
    o = np.zeros((BSZ, DIM), np.float32)
    for r in res.results:
        part_t = r["part_t"].reshape(P, MT2, BSZ)
        o += part_t.transpose(2, 1, 0).reshape(BSZ, DIM)
    out = np.ascontiguousarray(
        np.broadcast_to(o[:, None, :], (BSZ, SEQ, DIM))
    ).astype(np.float32, copy=False)
    return out, res


def kernel(**inputs):
    out, _ = _run(inputs)
    return out
